# revision 1
# baseline (speedup 1.0000x reference)
"""Trainium2 Bass kernel for nn_Cross_At_50208167690358 (cosine-sim cross attention).

Math (per reference.py):
  q = x @ Wq + bq                       [B, HW, C]
  kv = y @ Wkv + bkv -> k, v            [B, H, HW, hd] each
  q, k l2-normalized over hd; attn = softmax((q_hat @ k_hat^T) * temp); out = attn @ v
  B=2, HW=4096, C=64, H=8, hd=8.

Sharding: 16 (b, h) units -> 2 per core (cores share batch b = core // 4).

Per-core pipeline (per (b,h) unit):
  Phase B: project q/k/v in natural layout ([128 i, 8 d] tiles) with x^T/y^T as
  stationary matmul weights (exact fp32), l2-normalize q,k cheaply in natural
  layout (temperature folded into k), PE-transpose q_hat/k_hat to [d, i] layout.
  k_hat lands in a "quad" layout (j-tile t on PE row-group t%4) so QK^T runs as
  row-packed concurrent K=8 float32r matmuls; q_hat is replicated to all 4 row
  groups. V is augmented with a ones column so the PV matmul also produces the
  softmax denominator.

  Main loop (per 512-wide i-chunk): for each group of 3 j-tiles: 3 packed QK^T
  matmuls -> S^T [128 j, 3, 512 i] PSUM -> single ACT exp -> P^T (float32r,
  SBUF) -> 3 PV matmuls accumulating PV^T[9, 512] over all j. Epilogue:
  PE-transpose PV^T back to natural layout, divide by the denominator, DMA out.

  exp dominates: ~33.5M exps/core on ScalarE; all Ln/normalization ACT work is
  hoisted before the exp stream so the activation table set never switches
  mid-stream.
"""

import os
import sys

if "/opt/trn_rl_repo" not in sys.path:
    sys.path.insert(0, "/opt/trn_rl_repo")

import numpy as np
from contextlib import ExitStack

import concourse.bass as bass  # noqa: F401  (engine types referenced via nc)
from concourse import bacc, mybir
import concourse.tile as tile
from concourse.bass_utils import run_bass_kernel_spmd
from concourse.masks import make_identity

P = 128
HW = 4096
C = 64
H = 8
D = 8          # head dim
B = 2
NCORES = 8
NU = 2         # (b, h) units per core
NIT = HW // P  # 32 i-tiles / j-tiles
IC = 512       # i-chunk width (one PSUM bank of fp32)
NICH = HW // IC

F32 = mybir.dt.float32
F32R = mybir.dt.float32r
AF = mybir.ActivationFunctionType

# j-tile groups per exp instruction
GRPW = 2
GROUPS = [list(range(GRPW * g, GRPW * g + GRPW)) for g in range(NIT // GRPW)]

_CACHE = {}


def _emit_proj_norm(nc, pools, u, tensors):
    """Projections + l2 normalization for unit u (all ACT Ln/Exp-scale here)."""
    unitbuf, ring, work = pools["unit"], pools["ring"], pools["work"]
    xT, yT, wq, wk, tempb = (tensors["xT"], tensors["yT"], tensors["wq"],
                             tensors["wk"], tensors["tempb"])

    for name, shape, dt_, tag in (
            ("qrep", [P, HW], F32R, "qrep"),
            ("kL", [P, NIT // 4, P], F32R, "kL"),
            ("vaug", [P, NIT, D + 1], F32R, "vaug"),
            ("outsb", [P, NIT, D], F32, "outsb")):
        tensors[f"{name}{u}"] = unitbuf.tile(shape, dt_, tag=tag,
                                             name=f"{name}{u}")

    def project(w_ap, ncols, src):
        ps = ring.tile([P, NIT, ncols], F32, tag="ring")
        for it in range(NIT):
            nc.tensor.matmul(
                ps[:, it, :], src[:, it * P:(it + 1) * P], w_ap,
                start=True, stop=True)
        return ps

    def normalize(ps, apply_temp, tag):
        # ps: [P, NIT, D] natural-layout projection in PSUM (fp32).
        nat = work.tile([P, NIT, D], F32, tag="nat")
        nc.vector.tensor_copy(nat[:], ps[:])
        sq = work.tile([P, NIT, D], F32, tag="sq")
        nc.vector.tensor_mul(sq[:], nat[:], nat[:])
        ssum = work.tile([P, NIT], F32, tag="ssum")
        nc.vector.tensor_reduce(ssum[:], sq[:], mybir.AxisListType.X,
                                mybir.AluOpType.add)
        lns = work.tile([P, NIT], F32, tag="lns")
        nc.scalar.activation(lns[:], ssum[:], AF.Ln)
        inv = work.tile([P, NIT], F32, tag="inv")
        # 1/sqrt(s) = exp(-0.5 * ln(s))
        nc.scalar.activation(inv[:], lns[:], AF.Exp, scale=-0.5)
        if apply_temp:
            nc.vector.tensor_mul(
                inv[:], inv[:],
                tempb[:, u:u + 1].to_broadcast((P, NIT)))
        nhat = work.tile([P, NIT, D], F32, tag=tag)
        nc.vector.tensor_mul(
            nhat[:], nat[:], inv[:, :, None].to_broadcast((P, NIT, D)))
        return nhat

    ps_k = project(wk[:, u, :], D, yT)
    tensors[f"khat{u}"] = normalize(ps_k, True, f"khat{u}")
    ps_q = project(wq[:, u, :], D, xT)
    tensors[f"qhat{u}"] = normalize(ps_q, False, f"qhat{u}")


def _emit_layouts(nc, pools, u, tensors):
    """Transposes + layout DMAs + V projection for unit u (no ACT work).
    Generator: yields at chunk boundaries for interleaving."""
    ring, work = pools["ring"], pools["work"]
    yT, wv, ident = tensors["yT"], tensors["wv"], tensors["ident"]
    qrep, kL, vaug = (tensors[f"qrep{u}"], tensors[f"kL{u}"],
                      tensors[f"vaug{u}"])
    khat, qhat = tensors[f"khat{u}"], tensors[f"qhat{u}"]

    def transpose_rounds(nhat, dest):
        for r4 in range(NIT // 4):
            trq = ring.tile([D, 4, P], F32, tag="ring")
            for s in range(4):
                nc.tensor.transpose(trq[:, s, :], nhat[:, 4 * r4 + s, :], ident)
            nc.vector.tensor_copy(
                dest[0:D, 4 * r4 * P:(4 * r4 + 4) * P], trq[:])

    kT8 = work.tile([D, HW], F32R, tag="kT8")
    transpose_rounds(khat, kT8)
    yield
    # quad layout: j-tile t lands on PE row-group t%4 at kL[:, t//4, :]
    kT8v = kT8[:].rearrange("d (q g jj) -> d q g jj", g=4, jj=P)
    for g in range(4):
        nc.sync.dma_start(kL[32 * g:32 * g + D, :, :], kT8v[:, :, g, :])
    yield
    transpose_rounds(qhat, qrep)
    yield
    for g in range(1, 4):
        nc.sync.dma_start(qrep[32 * g:32 * g + D, :], qrep[0:D, :])
    yield
    ps_v = ring.tile([P, NIT, D + 1], F32, tag="ring")
    for it in range(NIT):
        nc.tensor.matmul(
            ps_v[:, it, :], yT[:, it * P:(it + 1) * P], wv[:, u, :],
            start=True, stop=True)
    nc.vector.tensor_copy(vaug[:], ps_v[:])
    yield


def _emit_main_ic(nc, pools, u, ic, tensors):
    """Main attention loop for one 512-wide i-chunk of unit u."""
    work, st_pool, pv_pool = pools["work"], pools["st"], pools["pv"]
    qrep, kL, vaug, outsb, ident = (
        tensors[f"qrep{u}"], tensors[f"kL{u}"], tensors[f"vaug{u}"],
        tensors[f"outsb{u}"], tensors["ident"])

    pvt = pv_pool.tile([D + 1, IC], F32, tag="pv")
    for jts in GROUPS:
        n = len(jts)
        st = st_pool.tile([P, GRPW, IC], F32, tag="st")
        for x, t in enumerate(jts):
            g = t % 4
            nc.tensor.matmul(
                st[:, x, :],
                kL[32 * g:32 * g + D, t // 4, :],
                qrep[32 * g:32 * g + D, ic * IC:(ic + 1) * IC],
                start=True, stop=True,
                tile_position=(32 * g, 0))
        pt = pools["pt"].tile([P, GRPW, IC], F32R, tag="pt")
        if os.environ.get("ABLATE") == "noexp":
            # timing experiment: token 1-column exp keeps the dependency
            # structure but removes ~all ACT work (results are garbage)
            nc.scalar.activation(pt[:, :n, 0:1], st[:, :n, 0:1], AF.Exp)
        else:
            nc.scalar.activation(pt[:, :n, :], st[:, :n, :], AF.Exp)
        for x, t in enumerate(jts):
            nc.tensor.matmul(
                pvt[:], vaug[:, t, :], pt[:, x, :],
                start=(t == 0), stop=(t == NIT - 1))
    # epilogue: PV^T [9, 512] -> natural [128, 4, 9], divide by denominator
    pv_sb = work.tile([D + 1, IC], F32, tag="pvsb")
    nc.vector.tensor_copy(pv_sb[:], pvt[:])
    trp = pools["tr"].tile([P, 4, D + 1], F32, tag="tr")
    for s in range(4):
        nc.tensor.transpose(
            trp[:, s, :], pv_sb[:, s * P:(s + 1) * P],
            ident[0:D + 1, 0:D + 1])
    rsum = work.tile([P, 4, 1], F32, tag="rsum")
    nc.vector.reciprocal(rsum[:], trp[:, :, D:D + 1])
    nc.vector.tensor_mul(
        outsb[:, ic * 4:(ic + 1) * 4, :], trp[:, :, 0:D],
        rsum[:].to_broadcast((P, 4, D)))


def build_program(reps=1):
    """reps>1 wraps the whole kernel in an on-device For_i loop (for timing
    runs: amortizes the ~90ms host/axon dispatch overhead)."""
    nc = bacc.Bacc("TRN2", target_bir_lowering=False, debug=False,
                   num_devices=NCORES)
    xT_d = nc.dram_tensor("xT_aug", [C + 1, HW], F32, kind="ExternalInput").ap()
    yT_d = nc.dram_tensor("yT_aug", [C + 1, HW], F32, kind="ExternalInput").ap()
    wq_d = nc.dram_tensor("wq_aug", [C + 1, NU, D], F32, kind="ExternalInput").ap()
    wk_d = nc.dram_tensor("wk_aug", [C + 1, NU, D], F32, kind="ExternalInput").ap()
    wv_d = nc.dram_tensor("wv_aug", [C + 1, NU, D + 1], F32, kind="ExternalInput").ap()
    temp_d = nc.dram_tensor("temp", [NU, 1], F32, kind="ExternalInput").ap()
    out_d = nc.dram_tensor("out", [NU, HW, D], F32, kind="ExternalOutput").ap()

    with tile.TileContext(nc) as tc, ExitStack() as ctx:
        pools = {
            "const": ctx.enter_context(tc.tile_pool(name="const", bufs=1)),
            "unit": ctx.enter_context(tc.tile_pool(name="unit", bufs=2)),
            # PSUM budget (8 banks): ring 2 + st 2x2 + pv 1 + tr 1 = 8
            "ring": ctx.enter_context(
                tc.tile_pool(name="ring", bufs=2, space="PSUM")),
            "st": ctx.enter_context(
                tc.tile_pool(name="st", bufs=2, space="PSUM")),
            "pv": ctx.enter_context(
                tc.tile_pool(name="pv", bufs=1, space="PSUM")),
            "tr": ctx.enter_context(
                tc.tile_pool(name="tr", bufs=1, space="PSUM")),
            "work": ctx.enter_context(tc.tile_pool(name="work", bufs=2)),
            "pt": ctx.enter_context(tc.tile_pool(name="pt", bufs=4)),
        }

        def emit_all():
            const = pools["const"]
            xT = const.tile([C + 1, HW], F32)
            yT = const.tile([C + 1, HW], F32)
            wq = const.tile([C + 1, NU, D], F32)
            wk = const.tile([C + 1, NU, D], F32)
            wv = const.tile([C + 1, NU, D + 1], F32)
            tempb = const.tile([P, NU], F32)
            ident = const.tile([P, P], F32)
            nc.sync.dma_start(xT[:], xT_d)
            nc.sync.dma_start(yT[:], yT_d)
            nc.sync.dma_start(wq[:], wq_d)
            nc.sync.dma_start(wk[:], wk_d)
            nc.sync.dma_start(wv[:], wv_d)
            for u in range(NU):
                nc.sync.dma_start(
                    tempb[:, u:u + 1], temp_d[u:u + 1, :].to_broadcast((P, 1)))
            make_identity(nc, ident[:])

            tensors = {"xT": xT, "yT": yT, "wq": wq, "wk": wk, "wv": wv,
                       "tempb": tempb, "ident": ident}

            # All Ln/normalize ACT work first (single table-set era), then
            # unit 0 layouts, then mains with unit 1 layout chunks
            # interleaved (keeps PE busy under the ACT-bound exp stream).
            _emit_proj_norm(nc, pools, 0, tensors)
            _emit_proj_norm(nc, pools, 1, tensors)
            for _ in _emit_layouts(nc, pools, 0, tensors):
                pass
            lay1 = _emit_layouts(nc, pools, 1, tensors)
            for ic in range(NICH):
                _emit_main_ic(nc, pools, 0, ic, tensors)
                next(lay1, None)
            for _ in lay1:
                pass
            for ic in range(NICH):
                _emit_main_ic(nc, pools, 1, ic, tensors)

            for u in range(NU):
                nc.sync.dma_start(
                    out_d[u].rearrange("(it ii) d -> ii it d", ii=P),
                    tensors[f"outsb{u}"][:])

        if reps == 1:
            emit_all()
        else:
            with tc.For_i(0, reps, 1):
                emit_all()

    nc.compile()
    return nc


def _prep_inputs(x, y, Wq, bq, Wkv, bkv, temperature):
    """Host-side sharding: pure relayout/slicing of the inputs."""
    ones = np.ones((1, HW), dtype=np.float32)
    in_maps = []
    for c in range(NCORES):
        b = c // 4
        heads = [2 * (c % 4), 2 * (c % 4) + 1]
        xT_aug = np.concatenate([np.ascontiguousarray(x[b].T), ones], axis=0)
        yT_aug = np.concatenate([np.ascontiguousarray(y[b].T), ones], axis=0)
        wq_aug = np.empty((C + 1, NU, D), dtype=np.float32)
        wk_aug = np.empty((C + 1, NU, D), dtype=np.float32)
        wv_aug = np.zeros((C + 1, NU, D + 1), dtype=np.float32)
        temp = np.empty((NU, 1), dtype=np.float32)
        for u, h in enumerate(heads):
            wq_aug[:C, u, :] = Wq[:, D * h:D * (h + 1)]
            wq_aug[C, u, :] = bq[D * h:D * (h + 1)]
            wk_aug[:C, u, :] = Wkv[:, D * h:D * (h + 1)]
            wk_aug[C, u, :] = bkv[D * h:D * (h + 1)]
            wv_aug[:C, u, :D] = Wkv[:, C + D * h:C + D * (h + 1)]
            wv_aug[C, u, :D] = bkv[C + D * h:C + D * (h + 1)]
            wv_aug[C, u, D] = 1.0
            temp[u, 0] = np.asarray(temperature).reshape(H)[h]
        in_maps.append({
            "xT_aug": xT_aug, "yT_aug": yT_aug, "wq_aug": wq_aug,
            "wk_aug": wk_aug, "wv_aug": wv_aug, "temp": temp,
        })
    return in_maps


def run(x, y, Wq, bq, Wkv, bkv, temperature, trace=False):
    if "nc" not in _CACHE:
        _CACHE["nc"] = build_program()
    nc = _CACHE["nc"]
    in_maps = _prep_inputs(x, y, Wq, bq, Wkv, bkv, temperature)
    res = run_bass_kernel_spmd(nc, in_maps, core_ids=list(range(NCORES)),
                               trace=trace)
    out = np.empty((B, HW, C), dtype=np.float32)
    for c in range(NCORES):
        b = c // 4
        heads = [2 * (c % 4), 2 * (c % 4) + 1]
        core_out = res.results[c]["out"]
        for u, h in enumerate(heads):
            out[b, :, D * h:D * (h + 1)] = core_out[u]
    return out, res


def kernel(x, y, Wq, bq, Wkv, bkv, temperature):
    out, _ = run(np.asarray(x), np.asarray(y), np.asarray(Wq), np.asarray(bq),
                 np.asarray(Wkv), np.asarray(bkv), np.asarray(temperature))
    return out



# revision 2
# speedup vs baseline: 1.1026x; 1.1026x over previous
"""Trainium2 Bass kernel for nn_Cross_At_50208167690358 (cosine-sim cross attention).

Math (per reference.py):
  q = x @ Wq + bq                       [B, HW, C]
  kv = y @ Wkv + bkv -> k, v            [B, H, HW, hd] each
  q, k l2-normalized over hd; attn = softmax((q_hat @ k_hat^T) * temp); out = attn @ v
  B=2, HW=4096, C=64, H=8, hd=8.

Sharding: 16 (b, h) units -> 2 per core (cores share batch b = core // 4).

v6 (ACT-saturating stream):
  - fp16 matmul operands everywhere in the main loop (1 cyc/col at any PE
    pstate); exact fp32 projections + normalization.
  - quad kL layout (j-tile t on PE row-group t%4) so consecutive QK matmuls
    load weights into different PE quadrants (ldweights overlaps compute);
    q_hat replicated to all 4 row groups.
  - st PSUM pool has bufs=3: the WAR release chain QK(s+2) <- exp(s) becomes
    QK(s+3) <- exp(s), giving ~3 exp-periods of slack so HW semaphore
    latency never gates the ACT exp stream (the critical resource:
    ~262k ACT cycles/core).
  - No per-chunk PSUM transpose: PV^T chunks are copied to an SBUF outT
    [9, HW]; the transpose back to natural layout + divide by the softmax
    denominator runs as a tail phase after the exp stream, using the ring
    bank. PSUM budget: ring 1 + st 3x2 + pv 1 = 8 banks.
  - Single merged Ln and Exp instructions for all 4 normalizations (2 table
    loads per rep, none mid-stream).
"""

import os
import sys

if "/opt/trn_rl_repo" not in sys.path:
    sys.path.insert(0, "/opt/trn_rl_repo")

import numpy as np
from contextlib import ExitStack

import concourse.bass as bass  # noqa: F401
from concourse import bacc, mybir
import concourse.tile as tile
from concourse.bass_utils import run_bass_kernel_spmd
from concourse.masks import make_identity

P = 128
HW = 4096
C = 64
H = 8
D = 8          # head dim
B = 2
NCORES = 8
NU = 2         # (b, h) units per core
NIT = HW // P  # 32 j-tiles
IC = 512       # i-chunk width
NICH = HW // IC

F32 = mybir.dt.float32
F16 = mybir.dt.float16
AF = mybir.ActivationFunctionType

GRPW = 2
GROUPS = [list(range(GRPW * g, GRPW * g + GRPW)) for g in range(NIT // GRPW)]

_CACHE = {}


def _emit_proj_norm_all(nc, pools, tensors):
    """Projections (exact fp32) + l2 normalization for both units.
    One merged Ln and one merged Exp instruction."""
    unitbuf, ring, work = pools["unit"], pools["ring"], pools["work"]
    xT, yT, wq, wk, tempb = (tensors["xT"], tensors["yT"], tensors["wq"],
                             tensors["wk"], tensors["tempb"])

    for u in range(NU):
        for name, shape, dt_, tag in (
                ("qrep", [P, HW], F16, "qrep"),
                ("kL", [P, NIT // 4, P], F16, "kL"),
                ("vaug", [P, NIT, D + 1], F16, "vaug"),
                ("outT", [D + 1, NICH, IC], F32, "outT"),
                ("outsb", [P, NIT, D], F32, "outsb")):
            tensors[f"{name}{u}"] = unitbuf.tile(shape, dt_, tag=tag,
                                                 name=f"{name}{u}")

    ssum_all = work.tile([P, 2 * NU, NIT], F32, tag="ssum_all",
                         name="ssum_all")
    nats = {}
    for u in range(NU):
        for kidx, (kind, src, w) in enumerate(
                (("k", yT, wk), ("q", xT, wq))):
            ps = ring.tile([P, NIT, D], F32, tag="ring", name=f"ps{kind}{u}")
            for it in range(NIT):
                nc.tensor.matmul(
                    ps[:, it, :], src[:, it * P:(it + 1) * P], w[:, u, :],
                    start=True, stop=True)
            nat = work.tile([P, NIT, D], F32, tag=f"nat{u}{kind}",
                            name=f"nat{u}{kind}")
            nc.vector.tensor_copy(nat[:], ps[:])
            sq = work.tile([P, NIT, D], F32, tag="sq", name=f"sq{u}{kind}")
            nc.vector.tensor_mul(sq[:], nat[:], nat[:])
            nc.vector.tensor_reduce(
                ssum_all[:, 2 * u + kidx, :], sq[:], mybir.AxisListType.X,
                mybir.AluOpType.add)
            nats[(u, kind)] = nat

    lns = work.tile([P, 2 * NU, NIT], F32, tag="lns", name="lns")
    nc.scalar.activation(lns[:], ssum_all[:], AF.Ln)
    inv = work.tile([P, 2 * NU, NIT], F32, tag="inv", name="inv")
    # 1/sqrt(s) = exp(-0.5 * ln(s))
    nc.scalar.activation(inv[:], lns[:], AF.Exp, scale=-0.5)

    for u in range(NU):
        # temperature folded into k_hat
        nc.vector.tensor_mul(
            inv[:, 2 * u, :], inv[:, 2 * u, :],
            tempb[:, u:u + 1].to_broadcast((P, NIT)))
        for kidx, kind in enumerate(("k", "q")):
            nhat = work.tile([P, NIT, D], F32, tag=f"nhat{u}{kind}",
                             name=f"nhat{u}{kind}")
            nc.vector.tensor_mul(
                nhat[:], nats[(u, kind)][:],
                inv[:, 2 * u + kidx, :][:, :, None].to_broadcast((P, NIT, D)))
            tensors[f"{kind}hat{u}"] = nhat


def _emit_layouts(nc, pools, u, tensors):
    """Transposes + layout DMAs + V projection for unit u (no ACT work).
    Generator: yields for interleaving."""
    ring, work = pools["ring"], pools["work"]
    yT, wv, ident = tensors["yT"], tensors["wv"], tensors["ident"]
    qrep, kL, vaug = (tensors[f"qrep{u}"], tensors[f"kL{u}"],
                      tensors[f"vaug{u}"])
    khat, qhat = tensors[f"khat{u}"], tensors[f"qhat{u}"]

    def transpose_rounds(nhat, dest):
        for r4 in range(NIT // 4):
            trq = ring.tile([D, 4, P], F32, tag="ring", name="trq")
            for s in range(4):
                nc.tensor.transpose(trq[:, s, :], nhat[:, 4 * r4 + s, :], ident)
            nc.vector.tensor_copy(
                dest[0:D, 4 * r4 * P:(4 * r4 + 4) * P], trq[:])

    kT8 = work.tile([D, HW], F16, tag="kT8", name=f"kT8_{u}")
    transpose_rounds(khat, kT8)
    yield
    # quad layout: j-tile t lands on PE row-group t%4 at kL[:, t//4, :]
    kT8v = kT8[:].rearrange("d (q g jj) -> d q g jj", g=4, jj=P)
    for g in range(4):
        nc.sync.dma_start(kL[32 * g:32 * g + D, :, :], kT8v[:, :, g, :])
    yield
    transpose_rounds(qhat, qrep)
    yield
    for g in range(1, 4):
        nc.sync.dma_start(qrep[32 * g:32 * g + D, :], qrep[0:D, :])
    yield
    ps_v = ring.tile([P, NIT, D + 1], F32, tag="ring", name=f"psv{u}")
    for it in range(NIT):
        nc.tensor.matmul(
            ps_v[:, it, :], yT[:, it * P:(it + 1) * P], wv[:, u, :],
            start=True, stop=True)
    nc.vector.tensor_copy(vaug[:], ps_v[:])
    yield


def _emit_main_unit(nc, pools, u, tensors, lay):
    """Main loop for unit u: two independent chunk streams (ic, ic+4)
    interleaved at group granularity so each stream's QK->exp->PV latency
    hides under the other stream's exp instruction."""
    st_pool, pv_pool, pt_pool = pools["st"], pools["pv"], pools["pt"]
    qrep, kL, vaug = (tensors[f"qrep{u}"], tensors[f"kL{u}"],
                      tensors[f"vaug{u}"])
    ablate = os.environ.get("ABLATE")
    half = NICH // 2

    for icp in range(half):
        ics = (icp, icp + half)
        pvts = {ic: pv_pool.tile([D + 1, IC], F32, tag="pv", name="pvt")
                for ic in ics}
        for jts in GROUPS:
            for ic in ics:
                st = st_pool.tile([P, GRPW, IC], F32, tag="st", name="st")
                for x, t in enumerate(jts):
                    g = t % 4
                    nc.tensor.matmul(
                        st[:, x, :],
                        kL[32 * g:32 * g + D, t // 4, :],
                        qrep[32 * g:32 * g + D, ic * IC:(ic + 1) * IC],
                        start=True, stop=True,
                        tile_position=(32 * g, 0))
                pt = pt_pool.tile([P, GRPW, IC], F16, tag="pt", name="pt")
                if ablate == "noexp":
                    nc.scalar.activation(pt[:, :, 0:1], st[:, :, 0:1], AF.Exp)
                else:
                    nc.scalar.activation(pt[:], st[:], AF.Exp)
                for x, t in enumerate(jts):
                    nc.tensor.matmul(
                        pvts[ic][:], vaug[:, t, :], pt[:, x, :],
                        start=(t == 0), stop=(t == NIT - 1))
            if lay is not None:
                next(lay, None)
        for ic in ics:
            nc.vector.tensor_copy(tensors[f"outT{u}"][:, ic, :],
                                  pvts[ic][:])


def _emit_tail(nc, pools, u, tensors):
    """Transpose outT back to natural layout, divide by denominator."""
    ring, work = pools["ring"], pools["work"]
    ident = tensors["ident"]
    outT, outsb = tensors[f"outT{u}"], tensors[f"outsb{u}"]
    outTv = outT[:].rearrange("d nich (s p) -> d (nich s) p", p=P)
    for r4 in range(NIT // 4):
        trp = ring.tile([P, 4, D + 1], F32, tag="ring", name="trp")
        for s in range(4):
            nc.tensor.transpose(
                trp[:, s, :], outTv[:, 4 * r4 + s, :],
                ident[0:D + 1, 0:D + 1])
        rsum = work.tile([P, 4, 1], F32, tag="rsum", name="rsum")
        nc.vector.reciprocal(rsum[:], trp[:, :, D:D + 1])
        nc.vector.tensor_mul(
            outsb[:, 4 * r4:4 * r4 + 4, :], trp[:, :, 0:D],
            rsum[:].to_broadcast((P, 4, D)))


def build_program(reps=1):
    nc = bacc.Bacc("TRN2", target_bir_lowering=False, debug=False,
                   num_devices=NCORES)
    xT_d = nc.dram_tensor("xT_aug", [C + 1, HW], F32, kind="ExternalInput").ap()
    yT_d = nc.dram_tensor("yT_aug", [C + 1, HW], F32, kind="ExternalInput").ap()
    wq_d = nc.dram_tensor("wq_aug", [C + 1, NU, D], F32, kind="ExternalInput").ap()
    wk_d = nc.dram_tensor("wk_aug", [C + 1, NU, D], F32, kind="ExternalInput").ap()
    wv_d = nc.dram_tensor("wv_aug", [C + 1, NU, D + 1], F32, kind="ExternalInput").ap()
    temp_d = nc.dram_tensor("temp", [NU, 1], F32, kind="ExternalInput").ap()
    out_d = nc.dram_tensor("out", [NU, HW, D], F32, kind="ExternalOutput").ap()

    with tile.TileContext(nc) as tc, ExitStack() as ctx:
        pools = {
            "const": ctx.enter_context(tc.tile_pool(name="const", bufs=1)),
            "unit": ctx.enter_context(tc.tile_pool(name="unit", bufs=2)),
            # PSUM budget (8 banks): ring 2 + st 2x2 + pv 2 = 8
            "ring": ctx.enter_context(
                tc.tile_pool(name="ring", bufs=2, space="PSUM")),
            "st": ctx.enter_context(
                tc.tile_pool(name="st", bufs=2, space="PSUM")),
            "pv": ctx.enter_context(
                tc.tile_pool(name="pv", bufs=2, space="PSUM")),
            "work": ctx.enter_context(tc.tile_pool(name="work", bufs=2)),
            "pt": ctx.enter_context(tc.tile_pool(name="pt", bufs=4)),
        }

        def emit_all():
            const = pools["const"]
            xT = const.tile([C + 1, HW], F32, name="xT")
            yT = const.tile([C + 1, HW], F32, name="yT")
            wq = const.tile([C + 1, NU, D], F32, name="wq")
            wk = const.tile([C + 1, NU, D], F32, name="wk")
            wv = const.tile([C + 1, NU, D + 1], F32, name="wv")
            tempb = const.tile([P, NU], F32, name="tempb")
            ident = const.tile([P, P], F32, name="ident")
            nc.sync.dma_start(xT[:], xT_d)
            nc.sync.dma_start(yT[:], yT_d)
            nc.sync.dma_start(wq[:], wq_d)
            nc.sync.dma_start(wk[:], wk_d)
            nc.sync.dma_start(wv[:], wv_d)
            for u in range(NU):
                nc.sync.dma_start(
                    tempb[:, u:u + 1], temp_d[u:u + 1, :].to_broadcast((P, 1)))
            make_identity(nc, ident[:])

            tensors = {"xT": xT, "yT": yT, "wq": wq, "wk": wk, "wv": wv,
                       "tempb": tempb, "ident": ident}

            _emit_proj_norm_all(nc, pools, tensors)
            for _ in _emit_layouts(nc, pools, 0, tensors):
                pass
            lay1 = _emit_layouts(nc, pools, 1, tensors)
            _emit_main_unit(nc, pools, 0, tensors, lay1)
            for _ in lay1:
                pass
            _emit_main_unit(nc, pools, 1, tensors, None)

            for u in range(NU):
                _emit_tail(nc, pools, u, tensors)
                nc.sync.dma_start(
                    out_d[u].rearrange("(it ii) d -> ii it d", ii=P),
                    tensors[f"outsb{u}"][:])

        if reps == 1:
            emit_all()
        else:
            with tc.For_i(0, reps, 1):
                emit_all()

    nc.compile()
    return nc


def _prep_inputs(x, y, Wq, bq, Wkv, bkv, temperature):
    """Host-side sharding: pure relayout/slicing of the inputs."""
    ones = np.ones((1, HW), dtype=np.float32)
    in_maps = []
    for c in range(NCORES):
        b = c // 4
        heads = [2 * (c % 4), 2 * (c % 4) + 1]
        xT_aug = np.concatenate([np.ascontiguousarray(x[b].T), ones], axis=0)
        yT_aug = np.concatenate([np.ascontiguousarray(y[b].T), ones], axis=0)
        wq_aug = np.empty((C + 1, NU, D), dtype=np.float32)
        wk_aug = np.empty((C + 1, NU, D), dtype=np.float32)
        wv_aug = np.zeros((C + 1, NU, D + 1), dtype=np.float32)
        temp = np.empty((NU, 1), dtype=np.float32)
        for u, h in enumerate(heads):
            wq_aug[:C, u, :] = Wq[:, D * h:D * (h + 1)]
            wq_aug[C, u, :] = bq[D * h:D * (h + 1)]
            wk_aug[:C, u, :] = Wkv[:, D * h:D * (h + 1)]
            wk_aug[C, u, :] = bkv[D * h:D * (h + 1)]
            wv_aug[:C, u, :D] = Wkv[:, C + D * h:C + D * (h + 1)]
            wv_aug[C, u, :D] = bkv[C + D * h:C + D * (h + 1)]
            wv_aug[C, u, D] = 1.0
            temp[u, 0] = np.asarray(temperature).reshape(H)[h]
        in_maps.append({
            "xT_aug": xT_aug, "yT_aug": yT_aug, "wq_aug": wq_aug,
            "wk_aug": wk_aug, "wv_aug": wv_aug, "temp": temp,
        })
    return in_maps


def run(x, y, Wq, bq, Wkv, bkv, temperature, trace=False):
    if "nc" not in _CACHE:
        _CACHE["nc"] = build_program()
    nc = _CACHE["nc"]
    in_maps = _prep_inputs(x, y, Wq, bq, Wkv, bkv, temperature)
    res = run_bass_kernel_spmd(nc, in_maps, core_ids=list(range(NCORES)),
                               trace=trace)
    out = np.empty((B, HW, C), dtype=np.float32)
    for c in range(NCORES):
        b = c // 4
        heads = [2 * (c % 4), 2 * (c % 4) + 1]
        core_out = res.results[c]["out"]
        for u, h in enumerate(heads):
            out[b, :, D * h:D * (h + 1)] = core_out[u]
    return out, res


def kernel(x, y, Wq, bq, Wkv, bkv, temperature):
    out, _ = run(np.asarray(x), np.asarray(y), np.asarray(Wq), np.asarray(bq),
                 np.asarray(Wkv), np.asarray(bkv), np.asarray(temperature))
    return out
